# revision 1
# baseline (speedup 1.0000x reference)
"""Banded local attention (ATTN_WIDTH=128) with exp-before-softmax, on 8 trn2 cores.

Reference math (per batch b, row q, full S=4096 columns):
    s      = Q K^T / 8
    a      = exp(s - rowmax(s))          # exact full-row max m1 required
    a_mask = a * band_mask               # keep j - i in [-64, 63]
    w      = softmax(a_mask)             # over all 4096 entries incl. zeros
    out    = w V

Reformulation (validated vs reference):
  - a_mask in [0, 1] so the second softmax needs no max shift:
        w_k = e^{a_k} / (sum_band e^{a_j} + (S - nb))
  - 256-wide window per 128-row q-tile, multiplicative 0/1 mask M:
        eg    = exp(exp(sw - m1) * M)      # masked lanes -> exp(0) = 1
        denom = sum_w eg + (S - 256)
        numer = eg @ V_win + (sum_all V - sum_win V)
        out   = numer / denom              # band-count nb cancels

Sharding: 8 cores = 4 batches x 2 query-halves of 2048 rows.

The scores are computed ONCE per q-tile as 4 PSUM pairs of [128, 1024]
(8 x 512-col float32r matmuls). K columns are reordered per core so that
the window of q-tile i sits at compile-time columns [128i, 128i+256)
regardless of the core's query offset (SPMD-uniform):
  cols [0, 2176)    = padded window slice (pads filled with real columns
                      borrowed from the tail so every k appears exactly once)
  cols [2176, 4096) = remaining columns (row max only)
The row max is order-invariant, so the reorder is free.

Engine split per q-tile:
  PE : 8 m1/window matmuls (f32r, 1 col/cycle), 2 eg transposes, 2 eg@V
  DVE: 4 pair reduce_max + negated combine, denom+recip, numer+CV add,
       final 1/denom scale
  ACT: exp1 (bias -m1, reads the window straight from score PSUM),
       exp2 (fused row-sum accum), eg->f32r cast copy
  GP : window mask multiply
"""

import sys

if "/opt/trn_rl_repo" not in sys.path:
    sys.path.insert(0, "/opt/trn_rl_repo")

from contextlib import ExitStack

import numpy as np

import concourse.bacc as bacc
import concourse.bass as bass
import concourse.tile as tile
from concourse import mybir
from concourse.bass_utils import run_bass_kernel_spmd

B, S, D = 4, 4096, 64
ATTN_WIDTH = 128
PAD = ATTN_WIDTH // 2          # 64
W = 2 * ATTN_WIDTH             # 256 window per q-tile
HALF = S // 2                  # 2048 rows per core
NT = HALF // 128               # 16 q-tiles per core
KSLICE = HALF + 2 * PAD        # 2176 window-slice columns
N_CORES = 8
F32 = mybir.dt.float32
F32R = mybir.dt.float32r
BF16 = mybir.dt.bfloat16

_CACHE = {}


def _round_f32r(x: np.ndarray) -> np.ndarray:
    """Round fp32 to float32r (11-bit mantissa, round-to-nearest) like walrus."""
    u = np.ascontiguousarray(x, dtype=np.float32).view(np.uint32)
    r = ((u.astype(np.uint64) + 0x800) & 0xFFFFF000).astype(np.uint32)
    return r.view(np.float32)


def _emit(ctx: ExitStack, tc, params):
    nc = tc.nc
    Exp = mybir.ActivationFunctionType.Exp
    mx = mybir.AluOpType.max

    const = ctx.enter_context(tc.tile_pool(name="const", bufs=1))
    work = ctx.enter_context(tc.tile_pool(name="work", bufs=2))
    outp = ctx.enter_context(tc.tile_pool(name="outp", bufs=3))
    ps_sc = ctx.enter_context(tc.tile_pool(name="ps_sc", bufs=3, space="PSUM"))
    ps_fv = ctx.enter_context(tc.tile_pool(name="ps_fv", bufs=2, space="PSUM"))

    qtr_s = const.tile([64, HALF], F32R)
    ktr_s = const.tile([64, S], F32R)
    vsr_s = const.tile([128, (NT + 1) * 64], F32R)
    ma_s = const.tile([128, 3 * W], F32)
    cvb_s = const.tile([128, NT * 64], F32)
    id_s = const.tile([128, 128], F32R)
    # issue order = first-use order: tile 0 needs qtr[:, :128] + all 4 ktr pairs
    nc.sync.dma_start(qtr_s[:, 0:128], params["qtr"][:, 0:128])
    for c in range(8):
        nc.sync.dma_start(
            ktr_s[:, 512 * c : 512 * (c + 1)],
            params["ktr"][:, 512 * c : 512 * (c + 1)],
        )
    nc.sync.dma_start(ma_s[:], params["ma"][:])
    nc.sync.dma_start(vsr_s[:, 0 : 8 * 64], params["vsr"][:, 0 : 8 * 64])
    nc.sync.dma_start(id_s[:], params["idf"][:])
    for c in range(3):
        nc.sync.dma_start(
            qtr_s[:, 128 + 640 * c : 128 + 640 * (c + 1)],
            params["qtr"][:, 128 + 640 * c : 128 + 640 * (c + 1)],
        )
    nc.sync.dma_start(
        vsr_s[:, 8 * 64 : (NT + 1) * 64], params["vsr"][:, 8 * 64 : (NT + 1) * 64]
    )
    nc.sync.dma_start(cvb_s[:], params["cvb"][:])
    out = params["out"]

    for i in range(NT):
        qtile_r = qtr_s[:, 128 * i : 128 * (i + 1)]
        wp = (128 * i) // 1024          # pair holding the window start
        lo = 128 * i - 1024 * wp        # window offset within pair wp
        cross = lo + W > 1024           # window spans pairs wp, wp+1
        order = [wp] + ([wp + 1] if cross else [])
        order += [p for p in range(4) if p not in order]

        pair_tiles = {}
        mp = work.tile([128, 4], F32, tag="mp", bufs=3)
        for j, p in enumerate(order):
            sc = ps_sc.tile([128, 1024], F32, tag="sc")
            pair_tiles[p] = sc
            base = 1024 * p
            nc.tensor.matmul(
                sc[:, 0:512], qtile_r, ktr_s[:, base : base + 512],
                start=True, stop=True,
            )
            nc.tensor.matmul(
                sc[:, 512:1024], qtile_r, ktr_s[:, base + 512 : base + 1024],
                start=True, stop=True,
            )
            nc.vector.reduce_max(
                mp[:, j : j + 1], sc[:], axis=mybir.AxisListType.X
            )
        nm1 = work.tile([128, 1], F32, tag="nm1")
        nc.vector.tensor_reduce(
            nm1[:], mp[:], axis=mybir.AxisListType.X, op=mx, negate=True
        )

        # --- ew = exp(window) off the critical chain (no m1 dependency);
        #     masked on GPSIMD early; m1 applied later as exp(-m1) scale ---
        ew = work.tile([128, W], F32, tag="ew", bufs=3)
        if not cross:
            nc.scalar.activation(ew[:], pair_tiles[wp][:, lo : lo + W], Exp)
        else:
            n1 = 1024 - lo
            nc.scalar.activation(ew[:, 0:n1], pair_tiles[wp][:, lo:1024], Exp)
            nc.scalar.activation(
                ew[:, n1:W], pair_tiles[wp + 1][:, 0 : W - n1], Exp
            )
        msel = 0 if i == 0 else (2 if i == NT - 1 else 1)
        ewm = work.tile([128, W], F32, tag="ewm", bufs=3)
        nc.gpsimd.tensor_tensor(
            out=ewm[:], in0=ew[:], in1=ma_s[:, W * msel : W * (msel + 1)],
            op=mybir.AluOpType.mult,
        )

        em1 = work.tile([128, 1], F32, tag="em1")
        nc.scalar.activation(em1[:], nm1[:], Exp)
        am = work.tile([128, W], F32, tag="am", bufs=3)
        nc.vector.tensor_scalar_mul(am[:], ewm[:], em1[:])
        eg = work.tile([128, W], F32R, tag="eg", bufs=3)
        seg = work.tile([128, 1], F32, tag="seg")
        nc.scalar.activation(eg[:], am[:], Exp, accum_out=seg[:])

        # denom = seg + (S - W);  rec = 1 / denom
        den = work.tile([128, 1], F32, tag="den")
        nc.gpsimd.tensor_scalar_add(den[:], seg[:], float(S - W))
        rec = work.tile([128, 1], F32, tag="rec")
        nc.vector.reciprocal(rec[:], den[:])

        # --- numer = eg @ V_win + CV_i (eg transposed on PE) ---
        fvt = ps_fv.tile([128, 320], F32, tag="fvt")
        egt_ps = fvt[:, 0:256].bitcast(F32R)
        numer = fvt[:, 256:320]
        nc.tensor.transpose(egt_ps[:, 0:128], eg[:, 0:128], id_s[:])
        nc.tensor.transpose(egt_ps[:, 128:256], eg[:, 128:256], id_s[:])
        egt = work.tile([128, W], F32R, tag="egt_sb", bufs=3)
        nc.scalar.copy(egt[:], egt_ps[:])
        nc.tensor.matmul(
            numer[:], egt[:, 0:128], vsr_s[:, 64 * i : 64 * i + 64],
            start=True, stop=False,
        )
        nc.tensor.matmul(
            numer[:], egt[:, 128:256], vsr_s[:, 64 * (i + 1) : 64 * (i + 1) + 64],
            start=False, stop=True,
        )

        ncv = work.tile([128, 64], F32, tag="ncv")
        nc.vector.tensor_tensor(
            out=ncv[:], in0=numer[:], in1=cvb_s[:, 64 * i : 64 * i + 64],
            op=mybir.AluOpType.add,
        )
        out_sb = outp.tile([128, 64], F32, tag="out_sb")
        nc.scalar.activation(
            out_sb[:], ncv[:], mybir.ActivationFunctionType.Copy,
            bias=0.0, scale=rec[:],
        )
        nc.sync.dma_start(out[128 * i : 128 * (i + 1), :], out_sb[:])


def build_program():
    nc = bacc.Bacc("TRN2", target_bir_lowering=False, debug=False)
    params = {
        "qtr": nc.declare_dram_parameter("qtr", [64, HALF], F32R, isOutput=False),
        "ktr": nc.declare_dram_parameter("ktr", [64, S], F32R, isOutput=False),
        "vsr": nc.declare_dram_parameter(
            "vsr", [128, (NT + 1) * 64], F32R, isOutput=False
        ),
        "ma": nc.declare_dram_parameter("ma", [128, 3 * W], F32, isOutput=False),
        "cvb": nc.declare_dram_parameter("cvb", [128, NT * 64], F32, isOutput=False),
        "idf": nc.declare_dram_parameter("idf", [128, 128], F32R, isOutput=False),
        "out": nc.declare_dram_parameter("out", [HALF, D], F32, isOutput=True),
    }
    with tile.TileContext(nc) as tc:
        with ExitStack() as ctx:
            _emit(ctx, tc, params)
    nc.compile()
    return nc


def make_in_maps(Q, K, V):
    """Full inputs -> list of 8 per-core input dicts."""
    Q = np.ascontiguousarray(np.asarray(Q, dtype=np.float32))
    K = np.ascontiguousarray(np.asarray(K, dtype=np.float32))
    V = np.ascontiguousarray(np.asarray(V, dtype=np.float32))

    idf = np.eye(128, dtype=np.float32)
    r = np.arange(128)[:, None]
    c = np.arange(W)[None, :]
    base_band = (c >= r) & (c < r + 128)

    in_maps = []
    for core in range(N_CORES):
        b, h = divmod(core, 2)
        off = h * HALF
        # fold the 1/sqrt(D) = 1/8 score scale into Q (exact: power of two)
        qt = np.ascontiguousarray(Q[b, off : off + HALF].T) * np.float32(0.125)
        qtr = _round_f32r(qt)

        # K column order: [window slice (pads borrowed from elsewhere) | rest]
        if h == 0:
            order = np.concatenate(
                [np.arange(2112, 2176), np.arange(0, 2112), np.arange(2176, S)]
            )
        else:
            order = np.concatenate(
                [np.arange(1984, S), np.arange(1920, 1984), np.arange(0, 1920)]
            )
        ktr = _round_f32r(np.ascontiguousarray(K[b].T[:, order]))

        Vpad = np.zeros((S + 2 * PAD, D), dtype=np.float32)
        Vpad[PAD : PAD + S] = V[b]
        vsl = Vpad[off : off + KSLICE]                                # [2176, 64]
        vsl_r = _round_f32r(vsl)
        vsr = np.ascontiguousarray(
            vsl_r.reshape(NT + 1, 128, D).transpose(1, 0, 2).reshape(
                128, (NT + 1) * 64
            )
        )

        # multiplicative masks: [tile0 | interior | tile15], each [128, 256]
        interior = base_band.astype(np.float32)
        m0 = interior
        m15 = interior
        if h == 0:  # global q-tile 0: need k >= 0  -> c >= 64
            m0 = (base_band & (c >= PAD)).astype(np.float32)
        else:  # global last tile: k < S -> c < 192
            m15 = (base_band & (c < 192)).astype(np.float32)
        ma = np.ascontiguousarray(np.concatenate([m0, interior, m15], axis=1))

        # CV_i = sum_all V (exact) - sum_window V_rounded, broadcast to rows
        sv = V[b].sum(axis=0, dtype=np.float32)
        cv = np.zeros((NT, 64), dtype=np.float32)
        for i in range(NT):
            cv[i] = sv - vsl_r[128 * i : 128 * i + W].sum(axis=0, dtype=np.float32)
        cvb = np.ascontiguousarray(
            np.broadcast_to(cv.reshape(1, NT * 64), (128, NT * 64))
        ).astype(np.float32)

        in_maps.append(
            {"qtr": qtr, "ktr": ktr, "vsr": vsr, "ma": ma, "cvb": cvb, "idf": idf}
        )
    return in_maps


def _get_program():
    if "nc" not in _CACHE:
        _CACHE["nc"] = build_program()
    return _CACHE["nc"]


def kernel(Q, K, V):
    nc = _get_program()
    in_maps = make_in_maps(Q, K, V)
    res = run_bass_kernel_spmd(nc, in_maps, list(range(N_CORES)))
    out = np.zeros((B, S, D), dtype=np.float32)
    for core in range(N_CORES):
        b, h = divmod(core, 2)
        out[b, h * HALF : (h + 1) * HALF] = res.results[core]["out"]
    return out



# revision 9
# speedup vs baseline: 1.1045x; 1.1045x over previous
"""Banded local attention (ATTN_WIDTH=128) with exp-before-softmax, 8 trn2 cores. v2.

Reference math (per batch b, row q, full S=4096 columns):
    s      = Q K^T / 8
    a      = exp(s - rowmax(s))          # full-row max m1 required
    a_mask = a * band_mask               # keep j - i in [-64, 63]
    w      = softmax(a_mask)             # over all 4096 entries incl. zeros
    out    = w V

v2 design vs the f32r baseline:
  - All matmuls in bf16 (1 col/cycle vs f32r's 2): validated rel err 2.8e-4.
  - The full-row max is a UNION of exact DVE reduce_max over ~2.4 of the 4
    [128,1024] score pairs and a beta=16 log-sum-exp computed by the ACT
    engine's free exp-accumulator on the remaining ~1.6 pairs.  LSE >= max
    always, overshoot <= ln(n)/16 (validated union rel err 1.9e-3), so the
    scan is split across two engines instead of serializing on DVE.
  - Window scores come from a separate host-gathered kwin tensor (window
    cols also appear in the ktr scan — duplicates don't change the max),
    so scan PSUM tiles are transient and the pipeline is deep.
  - Denominator via a ones-column appended to the V window; the CV
    (sum of off-window V) + (S-nb) constant enter as a rank-1 third matmul.
    No ACT accumulation or DVE adds on the output path.
  - exp1 (e^s, unbiased) and exp2 (e^a) are chunk-batched [128,512] ACT ops;
    the per-row e^{-m1} scale and band mask fold into one GPSIMD
    scalar_tensor_tensor op.

Sharding: 8 cores = 4 batches x 2 query-halves of 2048 rows, 16 q-tiles each.
"""

import sys

if "/opt/trn_rl_repo" not in sys.path:
    sys.path.insert(0, "/opt/trn_rl_repo")

from contextlib import ExitStack

import numpy as np
import ml_dtypes

import concourse.bacc as bacc
import concourse.bass as bass
import concourse.tile as tile
from concourse import mybir
from concourse.bass_utils import run_bass_kernel_spmd

B, S, D = 4, 4096, 64
ATTN_WIDTH = 128
PAD = ATTN_WIDTH // 2          # 64
W = 2 * ATTN_WIDTH             # 256 window per q-tile
HALF = S // 2                  # 2048 rows per core
NT = HALF // 128               # 16 q-tiles per core
N_CORES = 8
F32 = mybir.dt.float32
BF16 = mybir.dt.bfloat16
BF = ml_dtypes.bfloat16

BETA = 16.0
KAPPA2 = 3.6366                # median(rowmax / row-sigma) on N(0,1) data
R0_BIAS = 0.1
DSPLIT = 256                   # DVE's share of pair 3 (ACT lse takes the rest)

_CACHE = {}


def _emit(ctx: ExitStack, tc, params):
    nc = tc.nc
    Exp = mybir.ActivationFunctionType.Exp
    Ln = mybir.ActivationFunctionType.Ln
    mx = mybir.AluOpType.max
    mult = mybir.AluOpType.mult
    add = mybir.AluOpType.add
    X = mybir.AxisListType.X

    const = ctx.enter_context(tc.tile_pool(name="const", bufs=1))
    work = ctx.enter_context(tc.tile_pool(name="work", bufs=2))
    outp = ctx.enter_context(tc.tile_pool(name="outp", bufs=3))
    ps_sc = ctx.enter_context(tc.tile_pool(name="ps_sc", bufs=2, space="PSUM"))
    ps_wi = ctx.enter_context(tc.tile_pool(name="ps_wi", bufs=2, space="PSUM"))
    ps_tr = ctx.enter_context(tc.tile_pool(name="ps_tr", bufs=1, space="PSUM"))
    ps_nu = ctx.enter_context(tc.tile_pool(name="ps_nu", bufs=1, space="PSUM"))

    qtr_s = const.tile([64, HALF], BF16)
    ktr_s = const.tile([64, S], BF16)
    kwin_s = const.tile([64, NT * W], BF16)
    vwin_s = const.tile([128, NT * 130], BF16)
    cvr_s = const.tile([1, NT * 65], BF16)
    ones1_s = const.tile([1, 128], BF16)
    mask_s = const.tile([128, 3 * W], BF16)
    r0_s = const.tile([128, NT], F32)
    nbr0_s = const.tile([128, NT], F32)
    id_s = const.tile([128, 128], BF16)

    # DMA in first-use order
    nc.sync.dma_start(qtr_s[:, 0:128], params["qtr"][:, 0:128])
    for p in range(4):
        nc.sync.dma_start(
            ktr_s[:, 1024 * p : 1024 * (p + 1)],
            params["ktr"][:, 1024 * p : 1024 * (p + 1)],
        )
    nc.sync.dma_start(nbr0_s[:], params["nbr0"][:])
    nc.sync.dma_start(r0_s[:], params["r0c"][:])
    nc.sync.dma_start(kwin_s[:, 0:512], params["kwin"][:, 0:512])
    nc.sync.dma_start(mask_s[:], params["mask"][:])
    nc.sync.dma_start(id_s[:], params["idf"][:])
    nc.sync.dma_start(ones1_s[:], params["ones1"][:])
    nc.sync.dma_start(qtr_s[:, 128:HALF], params["qtr"][:, 128:HALF])
    for c in range(1, 8):
        nc.sync.dma_start(
            kwin_s[:, 512 * c : 512 * (c + 1)],
            params["kwin"][:, 512 * c : 512 * (c + 1)],
        )
    for c in range(4):
        nc.sync.dma_start(
            vwin_s[:, 520 * c : 520 * (c + 1)],
            params["vwin"][:, 520 * c : 520 * (c + 1)],
        )
    nc.sync.dma_start(cvr_s[:], params["cvr"][:])
    out = params["out"]

    for c in range(NT // 2):
        # per-chunk staging tiles (2 q-tiles per chunk)
        win = ps_wi.tile([128, 512], F32, tag="win")
        amT = ps_tr.tile([128, 256], F32, tag="amT")
        amT_bf = amT.bitcast(BF16)  # [128, 512] bf16 view of the PSUM tile
        accA = work.tile([128, 4], F32, tag="accA", bufs=3)
        mp = work.tile([128, 6], F32, tag="mp", bufs=3)
        asum = work.tile([128, 2], F32, tag="asum", bufs=3)
        lnA = work.tile([128, 2], F32, tag="lnA", bufs=3)
        rt = work.tile([128, 2], F32, tag="rt", bufs=3)
        m3 = work.tile([128, 2], F32, tag="m3", bufs=3)
        mxc = work.tile([128, 2], F32, tag="mxc", bufs=3)
        em1 = work.tile([128, 2], F32, tag="em1", bufs=3)
        ew = work.tile([128, 512], BF16, tag="ew", bufs=2)
        am = work.tile([128, 512], BF16, tag="am", bufs=2)
        egt = work.tile([128, 512], BF16, tag="egt", bufs=2)

        for jj in range(2):
            i = 2 * c + jj
            qt = qtr_s[:, 128 * i : 128 * (i + 1)]

            # window scores into the chunk tile
            nc.tensor.matmul(
                win[:, 256 * jj : 256 * (jj + 1)],
                qt, kwin_s[:, W * i : W * (i + 1)],
                start=True, stop=True,
            )
            # scan pairs: 0,1 -> DVE exact max; 2 -> ACT lse; 3 -> split
            for p in range(4):
                sc = ps_sc.tile([128, 1024], F32, tag="sc")
                nc.tensor.matmul(
                    sc[:, 0:512], qt, ktr_s[:, 1024 * p : 1024 * p + 512],
                    start=True, stop=True,
                )
                nc.tensor.matmul(
                    sc[:, 512:1024], qt, ktr_s[:, 1024 * p + 512 : 1024 * (p + 1)],
                    start=True, stop=True,
                )
                if p < 2:
                    nc.vector.reduce_max(
                        mp[:, 3 * jj + p : 3 * jj + p + 1], sc[:], axis=X
                    )
                elif p == 2:
                    lso = work.tile([128, 1024], BF16, tag="lso", bufs=2)
                    nc.scalar.activation(
                        lso[:], sc[:], Exp,
                        bias=nbr0_s[:, i : i + 1], scale=BETA,
                        accum_out=accA[:, 2 * jj : 2 * jj + 1],
                    )
                else:
                    nc.vector.reduce_max(
                        mp[:, 3 * jj + 2 : 3 * jj + 3], sc[:, 0:DSPLIT], axis=X
                    )
                    lso2 = work.tile([128, 1024 - DSPLIT], BF16, tag="lso2", bufs=2)
                    nc.scalar.activation(
                        lso2[:], sc[:, DSPLIT:1024], Exp,
                        bias=nbr0_s[:, i : i + 1], scale=BETA,
                        accum_out=accA[:, 2 * jj + 1 : 2 * jj + 2],
                    )
            # asum_jj = accA[2jj] + accA[2jj+1]
            nc.vector.tensor_tensor(
                out=asum[:, jj : jj + 1],
                in0=accA[:, 2 * jj : 2 * jj + 1],
                in1=accA[:, 2 * jj + 1 : 2 * jj + 2],
                op=add,
            )

        # r~ = r0 + ln(asum)/beta  (>= true max of the lse-covered cols)
        nc.scalar.activation(lnA[:], asum[:], Ln)
        nc.vector.scalar_tensor_tensor(
            out=rt[:], in0=lnA[:], scalar=1.0 / BETA,
            in1=r0_s[:, 2 * c : 2 * c + 2], op0=mult, op1=add,
        )
        # m1 = max(dve maxes, r~); em1 = exp(-m1)
        for jj in range(2):
            nc.vector.tensor_reduce(
                m3[:, jj : jj + 1], mp[:, 3 * jj : 3 * jj + 3], axis=X, op=mx
            )
        nc.vector.tensor_tensor(out=mxc[:], in0=m3[:], in1=rt[:], op=mx)
        nc.scalar.activation(em1[:], mxc[:], Exp, scale=-1.0)

        # ew = exp(s_win)  (unbiased, chunk-batched)
        nc.scalar.activation(ew[:], win[:], Exp)

        for jj in range(2):
            i = 2 * c + jj
            msel = 0 if i == 0 else (2 if i == NT - 1 else 1)
            # am = (ew * mask) * em1: mask on GPSIMD (no em1 dep), scale on DVE
            nc.gpsimd.tensor_tensor(
                out=am[:, 256 * jj : 256 * (jj + 1)],
                in0=ew[:, 256 * jj : 256 * (jj + 1)],
                in1=mask_s[:, W * msel : W * (msel + 1)],
                op=mult,
            )
            nc.vector.tensor_scalar_mul(
                am[:, 256 * jj : 256 * (jj + 1)],
                am[:, 256 * jj : 256 * (jj + 1)],
                em1[:, jj : jj + 1],
            )
            # transpose am halves onto PSUM (bf16 view)
            for hh in range(2):
                nc.tensor.transpose(
                    amT_bf[:, 256 * jj + 128 * hh : 256 * jj + 128 * (hh + 1)],
                    am[:, 256 * jj + 128 * hh : 256 * jj + 128 * (hh + 1)],
                    id_s[:],
                )

        # eg^T = exp(am^T), chunk-batched from PSUM
        nc.scalar.activation(egt[:], amT_bf[:], Exp)

        nu2 = ps_nu.tile([128, 130], F32, tag="nu")
        for jj in range(2):
            i = 2 * c + jj
            nu = nu2[:, 65 * jj : 65 * (jj + 1)]
            nc.tensor.matmul(
                nu[:], egt[:, 256 * jj : 256 * jj + 128],
                vwin_s[:, 130 * i : 130 * i + 65],
                start=True, stop=False,
            )
            nc.tensor.matmul(
                nu[:], egt[:, 256 * jj + 128 : 256 * jj + 256],
                vwin_s[:, 130 * i + 65 : 130 * i + 130],
                start=False, stop=False,
            )
            nc.tensor.matmul(
                nu[:], ones1_s[:], cvr_s[:, 65 * i : 65 * i + 65],
                start=False, stop=True,
            )
            rec = work.tile([128, 1], F32, tag="rec", bufs=3)
            nc.vector.reciprocal(rec[:], nu[:, 64:65])
            out_sb = outp.tile([128, 64], F32, tag="out_sb")
            nc.vector.tensor_scalar_mul(out_sb[:], nu[:, 0:64], rec[:])
            nc.sync.dma_start(out[128 * i : 128 * (i + 1), :], out_sb[:])


def build_program():
    nc = bacc.Bacc("TRN2", target_bir_lowering=False, debug=False)
    params = {
        "qtr": nc.declare_dram_parameter("qtr", [64, HALF], BF16, isOutput=False),
        "ktr": nc.declare_dram_parameter("ktr", [64, S], BF16, isOutput=False),
        "kwin": nc.declare_dram_parameter("kwin", [64, NT * W], BF16, isOutput=False),
        "vwin": nc.declare_dram_parameter(
            "vwin", [128, NT * 130], BF16, isOutput=False
        ),
        "cvr": nc.declare_dram_parameter("cvr", [1, NT * 65], BF16, isOutput=False),
        "ones1": nc.declare_dram_parameter("ones1", [1, 128], BF16, isOutput=False),
        "mask": nc.declare_dram_parameter("mask", [128, 3 * W], BF16, isOutput=False),
        "r0c": nc.declare_dram_parameter("r0c", [128, NT], F32, isOutput=False),
        "nbr0": nc.declare_dram_parameter("nbr0", [128, NT], F32, isOutput=False),
        "idf": nc.declare_dram_parameter("idf", [128, 128], BF16, isOutput=False),
        "out": nc.declare_dram_parameter("out", [HALF, D], F32, isOutput=True),
    }
    with tile.TileContext(nc) as tc:
        with ExitStack() as ctx:
            _emit(ctx, tc, params)
    nc.compile()
    return nc


def make_in_maps(Q, K, V):
    """Full inputs -> list of 8 per-core input dicts."""
    Q = np.ascontiguousarray(np.asarray(Q, dtype=np.float32))
    K = np.ascontiguousarray(np.asarray(K, dtype=np.float32))
    V = np.ascontiguousarray(np.asarray(V, dtype=np.float32))

    idf = np.eye(128, dtype=np.float32).astype(BF)
    ones1 = np.ones((1, 128), np.float32).astype(BF)
    r = np.arange(128)[:, None]
    cc = np.arange(W)[None, :]
    base_band = (cc >= r) & (cc <= r + 127)
    m_int = base_band.astype(np.float32)

    in_maps = []
    for core in range(N_CORES):
        b, h = divmod(core, 2)
        off = h * HALF
        # edge clamps only on the cores holding the global first/last rows
        m0 = (base_band & (cc >= PAD)).astype(np.float32) if h == 0 else m_int
        m15 = (base_band & (cc < 192)).astype(np.float32) if h == 1 else m_int
        mask3 = np.ascontiguousarray(
            np.concatenate([m0, m_int, m15], axis=1)
        ).astype(BF)
        qtr = np.ascontiguousarray(Q[b, off : off + HALF].T / 8.0).astype(BF)
        ktr = np.ascontiguousarray(K[b].T).astype(BF)

        Kpad = np.zeros((S + 2 * PAD, D), np.float32)
        Kpad[PAD : PAD + S] = K[b]
        Vpad = np.zeros((S + 2 * PAD, D), np.float32)
        Vpad[PAD : PAD + S] = V[b]
        valid = np.zeros((S + 2 * PAD,), np.float32)
        valid[PAD : PAD + S] = 1.0

        kwin = np.zeros((64, NT * W), np.float32)
        vwin = np.zeros((128, NT * 130), np.float32)
        cvr = np.zeros((1, NT * 65), np.float64)
        sv = V[b].astype(np.float64).sum(axis=0)
        for i in range(NT):
            lo = off + 128 * i  # padded index of window start
            kwin[:, W * i : W * (i + 1)] = Kpad[lo : lo + W].T
            wvalid = valid[lo : lo + W]
            wv_bf = Vpad[lo : lo + W].astype(BF).astype(np.float32)
            for hh in range(2):
                sl = slice(130 * i + 65 * hh, 130 * i + 65 * hh + 65)
                vwin[:, sl.start : sl.start + 64] = wv_bf[128 * hh : 128 * (hh + 1)]
                vwin[:, sl.start + 64] = wvalid[128 * hh : 128 * (hh + 1)]
            cvr[0, 65 * i : 65 * i + 64] = sv - wv_bf[wvalid > 0].astype(
                np.float64
            ).sum(axis=0)
            cvr[0, 65 * i + 64] = float(S) - wvalid.sum()

        # r0: per-row sigma from the exact second moment via G = K^T K
        G = (K[b].astype(np.float64).T @ K[b].astype(np.float64))
        Qb = Q[b, off : off + HALF].astype(np.float64)
        m2 = np.einsum("qd,de,qe->q", Qb, G, Qb)
        sig = np.sqrt(np.maximum(m2, 1e-12)) / (8.0 * 64.0)
        r0 = (KAPPA2 * sig + R0_BIAS).astype(np.float32)
        r0c = np.ascontiguousarray(r0.reshape(NT, 128).T)
        nbr0 = np.ascontiguousarray(-BETA * r0c)

        in_maps.append(
            {
                "qtr": qtr,
                "ktr": ktr,
                "kwin": np.ascontiguousarray(kwin).astype(BF),
                "vwin": np.ascontiguousarray(vwin).astype(BF),
                "cvr": np.ascontiguousarray(cvr.astype(np.float32)).astype(BF),
                "ones1": ones1,
                "mask": mask3,
                "r0c": r0c,
                "nbr0": nbr0,
                "idf": idf,
            }
        )
    return in_maps


def _get_program():
    if "nc" not in _CACHE:
        _CACHE["nc"] = build_program()
    return _CACHE["nc"]


def kernel(Q, K, V):
    nc = _get_program()
    in_maps = make_in_maps(Q, K, V)
    res = run_bass_kernel_spmd(nc, in_maps, list(range(N_CORES)))
    out = np.zeros((B, S, D), dtype=np.float32)
    for core in range(N_CORES):
        b, h = divmod(core, 2)
        out[b, h * HALF : (h + 1) * HALF] = res.results[core]["out"]
    return out


# revision 17
# speedup vs baseline: 1.1451x; 1.0368x over previous
"""Banded local attention (ATTN_WIDTH=128) with exp-before-softmax, 8 trn2 cores. v2.

Reference math (per batch b, row q, full S=4096 columns):
    s      = Q K^T / 8
    a      = exp(s - rowmax(s))          # full-row max m1 required
    a_mask = a * band_mask               # keep j - i in [-64, 63]
    w      = softmax(a_mask)             # over all 4096 entries incl. zeros
    out    = w V

v2 design vs the f32r baseline:
  - All matmuls in bf16 (1 col/cycle vs f32r's 2): validated rel err 2.8e-4.
  - The full-row max is a UNION of exact DVE reduce_max over ~2.4 of the 4
    [128,1024] score pairs and a beta=16 log-sum-exp computed by the ACT
    engine's free exp-accumulator on the remaining ~1.6 pairs.  LSE >= max
    always, overshoot <= ln(n)/16 (validated union rel err 1.9e-3), so the
    scan is split across two engines instead of serializing on DVE.
  - Window scores come from a separate host-gathered kwin tensor (window
    cols also appear in the ktr scan — duplicates don't change the max),
    so scan PSUM tiles are transient and the pipeline is deep.
  - Denominator via a ones-column appended to the V window; the CV
    (sum of off-window V) + (S-nb) constant enter as a rank-1 third matmul.
    No ACT accumulation or DVE adds on the output path.
  - exp1 (e^s, unbiased) and exp2 (e^a) are chunk-batched [128,512] ACT ops;
    the per-row e^{-m1} scale and band mask fold into one GPSIMD
    scalar_tensor_tensor op.

Sharding: 8 cores = 4 batches x 2 query-halves of 2048 rows, 16 q-tiles each.
"""

import sys

if "/opt/trn_rl_repo" not in sys.path:
    sys.path.insert(0, "/opt/trn_rl_repo")

from contextlib import ExitStack

import numpy as np
import ml_dtypes

import concourse.bacc as bacc
import concourse.bass as bass
import concourse.tile as tile
from concourse import mybir
from concourse.bass_utils import run_bass_kernel_spmd

B, S, D = 4, 4096, 64
ATTN_WIDTH = 128
PAD = ATTN_WIDTH // 2          # 64
W = 2 * ATTN_WIDTH             # 256 window per q-tile
HALF = S // 2                  # 2048 rows per core
NT = HALF // 128               # 16 q-tiles per core
N_CORES = 8
F32 = mybir.dt.float32
BF16 = mybir.dt.bfloat16
BF = ml_dtypes.bfloat16

BETA = 16.0
KAPPA2 = 3.6366                # median(rowmax / row-sigma) on N(0,1) data
R0_BIAS = 0.1
DSPLIT = 256                   # DVE's share of pair 3 (ACT lse takes the rest)

_CACHE = {}


def _emit(ctx: ExitStack, tc, params):
    nc = tc.nc
    Exp = mybir.ActivationFunctionType.Exp
    Ln = mybir.ActivationFunctionType.Ln
    mx = mybir.AluOpType.max
    mult = mybir.AluOpType.mult
    add = mybir.AluOpType.add
    X = mybir.AxisListType.X

    const = ctx.enter_context(tc.tile_pool(name="const", bufs=1))
    work = ctx.enter_context(tc.tile_pool(name="work", bufs=2))
    outp = ctx.enter_context(tc.tile_pool(name="outp", bufs=3))
    ps_sc = ctx.enter_context(tc.tile_pool(name="ps_sc", bufs=2, space="PSUM"))
    ps_wi = ctx.enter_context(tc.tile_pool(name="ps_wi", bufs=2, space="PSUM"))
    ps_tr = ctx.enter_context(tc.tile_pool(name="ps_tr", bufs=1, space="PSUM"))
    ps_nu = ctx.enter_context(tc.tile_pool(name="ps_nu", bufs=1, space="PSUM"))

    qtr_s = const.tile([64, HALF], BF16)
    ktr_s = const.tile([64, S], BF16)
    kwin_s = const.tile([64, NT * W], BF16)
    vwin_s = const.tile([128, NT * 130], BF16)
    cvr_s = const.tile([1, NT * 65], BF16)
    ones1_s = const.tile([1, 128], BF16)
    mask_s = const.tile([128, 3 * W], BF16)
    nr0_s = const.tile([128, NT], F32)
    nbr0_s = const.tile([128, NT], F32)
    id_s = const.tile([128, 128], BF16)

    # DMA in first-use order
    nc.sync.dma_start(qtr_s[:, 0:128], params["qtr"][:, 0:128])
    for p in range(4):
        nc.sync.dma_start(
            ktr_s[:, 1024 * p : 1024 * (p + 1)],
            params["ktr"][:, 1024 * p : 1024 * (p + 1)],
        )
    nc.sync.dma_start(nbr0_s[:], params["nbr0"][:])
    nc.sync.dma_start(nr0_s[:], params["nr0c"][:])
    nc.sync.dma_start(kwin_s[:, 0:512], params["kwin"][:, 0:512])
    nc.sync.dma_start(mask_s[:], params["mask"][:])
    nc.sync.dma_start(id_s[:], params["idf"][:])
    nc.sync.dma_start(ones1_s[:], params["ones1"][:])
    nc.sync.dma_start(qtr_s[:, 128:HALF], params["qtr"][:, 128:HALF])
    for c in range(1, 8):
        nc.sync.dma_start(
            kwin_s[:, 512 * c : 512 * (c + 1)],
            params["kwin"][:, 512 * c : 512 * (c + 1)],
        )
    for c in range(4):
        nc.sync.dma_start(
            vwin_s[:, 520 * c : 520 * (c + 1)],
            params["vwin"][:, 520 * c : 520 * (c + 1)],
        )
    nc.sync.dma_start(cvr_s[:], params["cvr"][:])
    out = params["out"]

    for c in range(NT // 2):
        # per-chunk staging tiles (2 q-tiles per chunk)
        win = ps_wi.tile([128, 512], F32, tag="win")
        amT = ps_tr.tile([128, 256], F32, tag="amT")
        amT_bf = amT.bitcast(BF16)  # [128, 512] bf16 view of the PSUM tile
        accA = work.tile([128, 2], F32, tag="accA", bufs=3)
        mp = work.tile([128, 6], F32, tag="mp", bufs=3)
        lnA = work.tile([128, 2], F32, tag="lnA", bufs=3)
        nrt = work.tile([128, 2], F32, tag="nrt", bufs=3)
        nm3 = work.tile([128, 2], F32, tag="nm3", bufs=3)
        nmx = work.tile([128, 2], F32, tag="nmx", bufs=3)
        ew = work.tile([128, 512], BF16, tag="ew", bufs=2)
        am = work.tile([128, 512], BF16, tag="am", bufs=2)
        egt = work.tile([128, 512], BF16, tag="egt", bufs=2)

        for jj in range(2):
            i = 2 * c + jj
            qt = qtr_s[:, 128 * i : 128 * (i + 1)]

            # window scores into the chunk tile
            nc.tensor.matmul(
                win[:, 256 * jj : 256 * (jj + 1)],
                qt, kwin_s[:, W * i : W * (i + 1)],
                start=True, stop=True,
            )
            # scan pairs: 0,1,2 -> DVE exact max; 3 -> ACT beta-lse
            for p in range(4):
                sc = ps_sc.tile([128, 1024], F32, tag="sc")
                nc.tensor.matmul(
                    sc[:, 0:512], qt, ktr_s[:, 1024 * p : 1024 * p + 512],
                    start=True, stop=True,
                )
                nc.tensor.matmul(
                    sc[:, 512:1024], qt, ktr_s[:, 1024 * p + 512 : 1024 * (p + 1)],
                    start=True, stop=True,
                )
                if p < 3:
                    nc.vector.reduce_max(
                        mp[:, 3 * jj + p : 3 * jj + p + 1], sc[:], axis=X
                    )
                else:
                    lso = work.tile([128, 1024], BF16, tag="lso", bufs=2)
                    nc.scalar.activation(
                        lso[:], sc[:], Exp,
                        bias=nbr0_s[:, i : i + 1], scale=BETA,
                        accum_out=accA[:, jj : jj + 1],
                    )

        # -r~ = -r0 - ln(A)/beta, with ln via the float bit-hack so the ACT
        # engine only ever needs the Exp table (no table-set thrash):
        #   ln(A) ~ (int_bits(A)*2^-23 - 126.94269504) * ln2   (err < 0.03)
        nc.vector.tensor_copy(lnA[:], accA.bitcast(mybir.dt.int32)[:])
        nc.vector.scalar_tensor_tensor(
            out=nrt[:], in0=lnA[:],
            scalar=-float(np.log(2.0) / (BETA * 2.0**23)),
            in1=nr0_s[:, 2 * c : 2 * c + 2], op0=mult, op1=add,
        )
        # -m1 = min(-dve_max, -r~)
        for jj in range(2):
            nc.vector.tensor_reduce(
                nm3[:, jj : jj + 1], mp[:, 3 * jj : 3 * jj + 3], axis=X, op=mx,
                negate=True,
            )
        nc.vector.tensor_tensor(
            out=nmx[:], in0=nm3[:], in1=nrt[:], op=mybir.AluOpType.min
        )

        for jj in range(2):
            i = 2 * c + jj
            msel = 0 if i == 0 else (2 if i == NT - 1 else 1)
            # ew = exp(s_win - m1)  (per-tile bias)
            nc.scalar.activation(
                ew[:, 256 * jj : 256 * (jj + 1)],
                win[:, 256 * jj : 256 * (jj + 1)],
                Exp, bias=nmx[:, jj : jj + 1],
            )
            # am = ew * mask on GPSIMD
            nc.gpsimd.tensor_tensor(
                out=am[:, 256 * jj : 256 * (jj + 1)],
                in0=ew[:, 256 * jj : 256 * (jj + 1)],
                in1=mask_s[:, W * msel : W * (msel + 1)],
                op=mult,
            )
            # transpose am halves onto PSUM (bf16 view)
            for hh in range(2):
                nc.tensor.transpose(
                    amT_bf[:, 256 * jj + 128 * hh : 256 * jj + 128 * (hh + 1)],
                    am[:, 256 * jj + 128 * hh : 256 * jj + 128 * (hh + 1)],
                    id_s[:],
                )

        # eg^T = exp(am^T), chunk-batched from PSUM
        nc.scalar.activation(egt[:], amT_bf[:], Exp)

        nu2 = ps_nu.tile([128, 130], F32, tag="nu")
        for jj in range(2):
            i = 2 * c + jj
            nu = nu2[:, 65 * jj : 65 * (jj + 1)]
            nc.tensor.matmul(
                nu[:], egt[:, 256 * jj : 256 * jj + 128],
                vwin_s[:, 130 * i : 130 * i + 65],
                start=True, stop=False,
            )
            nc.tensor.matmul(
                nu[:], egt[:, 256 * jj + 128 : 256 * jj + 256],
                vwin_s[:, 130 * i + 65 : 130 * i + 130],
                start=False, stop=False,
            )
            nc.tensor.matmul(
                nu[:], ones1_s[:], cvr_s[:, 65 * i : 65 * i + 65],
                start=False, stop=True,
            )
            rec = work.tile([128, 1], F32, tag="rec", bufs=3)
            nc.vector.reciprocal(rec[:], nu[:, 64:65])
            out_sb = outp.tile([128, 64], F32, tag="out_sb")
            nc.vector.tensor_scalar_mul(out_sb[:], nu[:, 0:64], rec[:])
            nc.sync.dma_start(out[128 * i : 128 * (i + 1), :], out_sb[:])


def _patch_act_tables():
    """Force Exp and Ln to share one table set (natural_log_exp_and_others).

    The default placement picks per-function favorite sets, which makes the
    per-chunk Exp/Ln alternation thrash ACT_TABLE_LOAD (~1.3us each)."""
    import concourse.hw_specs as hw_specs

    if getattr(bacc, "_act_tables_patched", False):
        return
    orig = hw_specs.get_activation_tables

    def filtered(arch):
        tabs = orig(arch)
        combined = {
            k: v for k, v in tabs.items() if k == "natural_log_exp_and_others"
        }
        return combined if combined else tabs

    bacc.get_activation_tables = filtered
    bacc._act_tables_patched = True


def build_program():
    # _patch_act_tables()  # isolating device failure
    nc = bacc.Bacc("TRN2", target_bir_lowering=False, debug=False)
    params = {
        "qtr": nc.declare_dram_parameter("qtr", [64, HALF], BF16, isOutput=False),
        "ktr": nc.declare_dram_parameter("ktr", [64, S], BF16, isOutput=False),
        "kwin": nc.declare_dram_parameter("kwin", [64, NT * W], BF16, isOutput=False),
        "vwin": nc.declare_dram_parameter(
            "vwin", [128, NT * 130], BF16, isOutput=False
        ),
        "cvr": nc.declare_dram_parameter("cvr", [1, NT * 65], BF16, isOutput=False),
        "ones1": nc.declare_dram_parameter("ones1", [1, 128], BF16, isOutput=False),
        "mask": nc.declare_dram_parameter("mask", [128, 3 * W], BF16, isOutput=False),
        "nr0c": nc.declare_dram_parameter("nr0c", [128, NT], F32, isOutput=False),
        "nbr0": nc.declare_dram_parameter("nbr0", [128, NT], F32, isOutput=False),
        "idf": nc.declare_dram_parameter("idf", [128, 128], BF16, isOutput=False),
        "out": nc.declare_dram_parameter("out", [HALF, D], F32, isOutput=True),
    }
    with tile.TileContext(nc) as tc:
        with ExitStack() as ctx:
            _emit(ctx, tc, params)
    nc.compile()
    return nc


def make_in_maps(Q, K, V):
    """Full inputs -> list of 8 per-core input dicts."""
    Q = np.ascontiguousarray(np.asarray(Q, dtype=np.float32))
    K = np.ascontiguousarray(np.asarray(K, dtype=np.float32))
    V = np.ascontiguousarray(np.asarray(V, dtype=np.float32))

    idf = np.eye(128, dtype=np.float32).astype(BF)
    ones1 = np.ones((1, 128), np.float32).astype(BF)
    r = np.arange(128)[:, None]
    cc = np.arange(W)[None, :]
    base_band = (cc >= r) & (cc <= r + 127)
    m_int = base_band.astype(np.float32)

    in_maps = []
    for core in range(N_CORES):
        b, h = divmod(core, 2)
        off = h * HALF
        # edge clamps only on the cores holding the global first/last rows
        m0 = (base_band & (cc >= PAD)).astype(np.float32) if h == 0 else m_int
        m15 = (base_band & (cc < 192)).astype(np.float32) if h == 1 else m_int
        mask3 = np.ascontiguousarray(
            np.concatenate([m0, m_int, m15], axis=1)
        ).astype(BF)
        qtr = np.ascontiguousarray(Q[b, off : off + HALF].T / 8.0).astype(BF)
        ktr = np.ascontiguousarray(K[b].T).astype(BF)

        Kpad = np.zeros((S + 2 * PAD, D), np.float32)
        Kpad[PAD : PAD + S] = K[b]
        Vpad = np.zeros((S + 2 * PAD, D), np.float32)
        Vpad[PAD : PAD + S] = V[b]
        valid = np.zeros((S + 2 * PAD,), np.float32)
        valid[PAD : PAD + S] = 1.0

        kwin = np.zeros((64, NT * W), np.float32)
        vwin = np.zeros((128, NT * 130), np.float32)
        cvr = np.zeros((1, NT * 65), np.float64)
        sv = V[b].astype(np.float64).sum(axis=0)
        for i in range(NT):
            lo = off + 128 * i  # padded index of window start
            kwin[:, W * i : W * (i + 1)] = Kpad[lo : lo + W].T
            wvalid = valid[lo : lo + W]
            wv_bf = Vpad[lo : lo + W].astype(BF).astype(np.float32)
            for hh in range(2):
                sl = slice(130 * i + 65 * hh, 130 * i + 65 * hh + 65)
                vwin[:, sl.start : sl.start + 64] = wv_bf[128 * hh : 128 * (hh + 1)]
                vwin[:, sl.start + 64] = wvalid[128 * hh : 128 * (hh + 1)]
            cvr[0, 65 * i : 65 * i + 64] = sv - wv_bf[wvalid > 0].astype(
                np.float64
            ).sum(axis=0)
            cvr[0, 65 * i + 64] = float(S) - wvalid.sum()

        # r0: per-row sigma from the exact second moment via G = K^T K
        G = (K[b].astype(np.float64).T @ K[b].astype(np.float64))
        Qb = Q[b, off : off + HALF].astype(np.float64)
        m2 = np.einsum("qd,de,qe->q", Qb, G, Qb)
        sig = np.sqrt(np.maximum(m2, 1e-12)) / (8.0 * 64.0)
        r0 = (KAPPA2 * sig + R0_BIAS).astype(np.float32)
        r0c = np.ascontiguousarray(r0.reshape(NT, 128).T)
        nbr0 = np.ascontiguousarray(-BETA * r0c)
        # -r~ = I*(-ln2/(beta*2^23)) + (126.94269504*ln2/beta - r0)
        nr0c = np.ascontiguousarray(
            (126.94269504 * np.log(2.0) / BETA - r0c).astype(np.float32)
        )

        in_maps.append(
            {
                "qtr": qtr,
                "ktr": ktr,
                "kwin": np.ascontiguousarray(kwin).astype(BF),
                "vwin": np.ascontiguousarray(vwin).astype(BF),
                "cvr": np.ascontiguousarray(cvr.astype(np.float32)).astype(BF),
                "ones1": ones1,
                "mask": mask3,
                "nr0c": nr0c,
                "nbr0": nbr0,
                "idf": idf,
            }
        )
    return in_maps


def _get_program():
    if "nc" not in _CACHE:
        _CACHE["nc"] = build_program()
    return _CACHE["nc"]


def kernel(Q, K, V):
    nc = _get_program()
    in_maps = make_in_maps(Q, K, V)
    res = run_bass_kernel_spmd(nc, in_maps, list(range(N_CORES)))
    out = np.zeros((B, S, D), dtype=np.float32)
    for core in range(N_CORES):
        b, h = divmod(core, 2)
        out[b, h * HALF : (h + 1) * HALF] = res.results[core]["out"]
    return out
